# revision 14
# baseline (speedup 1.0000x reference)
"""Trainium2 Bass kernel for nn_Block_48936857370743 (MLA transformer block).

Sharding: 8 cores = 2 batches x 4 contiguous token chunks of 512.
Weights are replicated per core (no collectives). Each core computes the
tiny z latent (L=64) for its batch's full sequence locally.

Layout: activations transposed (feature dim on partitions) through
attention; MLP output in natural layout.  rmsnorm is folded into
per-token scales: the q-side via a broadcast tile, the z-side via the
exp eviction's per-partition `scale=`, the MLP norm via per-partition
scales in natural layout.  Softmax denominators come from a ones-column
appended to z in the S matmul.  Causal mask is a per-core data input so
a single SPMD program serves all cores.
"""

import os
import sys

import numpy as np
import ml_dtypes

for _p in ("/opt/trn_rl_repo", "/root/.axon_site/_ro/trn_rl_repo"):
    if os.path.isdir(_p) and _p not in sys.path:
        sys.path.insert(0, _p)

import concourse.bacc as bacc
import concourse.mybir as mybir
from concourse import tile
from concourse.bass_utils import run_bass_kernel_spmd
from concourse.masks import make_identity

BF16 = mybir.dt.bfloat16
F32 = mybir.dt.float32
F32R = mybir.dt.float32r
AF = mybir.ActivationFunctionType

B, T, C, H = 2, 2048, 2048, 16
Dh, L, F = 128, 64, 8192
R = 512            # tokens per core
P = 128
NK = C // P        # 16 k-tiles over C
NF = F // P        # 64 tiles over F
NS = T // P        # 16 kv tiles
EPS = 5e-5
N_CORES = 8

nbf = ml_dtypes.bfloat16


def build_nc():
    nc = bacc.Bacc("TRN2", target_bir_lowering=False, debug=False,
                   num_devices=N_CORES)

    # ---- DRAM I/O ----
    d_xfull = nc.dram_tensor("xfull_bf", [C, T], BF16, kind="ExternalInput").ap()
    d_xown32 = nc.dram_tensor("xown_f32", [C, R], F32, kind="ExternalInput").ap()
    d_xownbf = nc.dram_tensor("xown_bf", [C, R], BF16, kind="ExternalInput").ap()
    d_mask = nc.dram_tensor("mask_bf", [T, R], BF16, kind="ExternalInput").ap()
    d_wq = nc.dram_tensor("wq_bf", [C, C], BF16, kind="ExternalInput").ap()
    d_wkv = nc.dram_tensor("wkv_bf", [C, L], BF16, kind="ExternalInput").ap()
    d_wkT = nc.dram_tensor("wkT_bf", [H, Dh, L], BF16, kind="ExternalInput").ap()
    d_wv = nc.dram_tensor("wv_bf", [H, L, Dh], BF16, kind="ExternalInput").ap()
    d_wo = nc.dram_tensor("wo_bf", [C, C], BF16, kind="ExternalInput").ap()
    d_wfc = nc.dram_tensor("wfc_bf", [C, F], BF16, kind="ExternalInput").ap()
    d_wproj = nc.dram_tensor("wproj_bf", [F, C], BF16, kind="ExternalInput").ap()
    d_out = nc.dram_tensor("out", [R, C], F32, kind="ExternalOutput").ap()

    with tile.TileContext(nc) as tc:
        with (
            tc.tile_pool(name="pers", bufs=1) as pers,
            tc.tile_pool(name="work", bufs=2) as work,
            tc.tile_pool(name="psum", bufs=1, space="PSUM") as pp,
        ):
            # ---------- constants ----------
            ones_col_bf = pers.tile([P, 1], BF16, tag="ones_col_bf",
                                    name="ones_col_bf")
            nc.vector.memset(ones_col_bf[:], 1.0)
            ones_row_bf = pers.tile([1, P], BF16, tag="ones_row_bf",
                                    name="ones_row_bf")
            nc.vector.memset(ones_row_bf[:], 1.0)
            ident = pers.tile([P, P], BF16, tag="ident", name="ident")
            make_identity(nc, ident[:])
            identf = pers.tile([P, P], F32, tag="identf", name="identf")
            make_identity(nc, identf[:])
            eps1 = pers.tile([1, 1], F32, tag="eps1", name="eps1")
            nc.vector.memset(eps1[:], EPS)
            epsc = pers.tile([P, 1], F32, tag="epsc", name="epsc")
            nc.vector.memset(epsc[:], EPS)

            # ---------- persistent loads (one DMA per tensor) ----------
            mask_all = pers.tile([P, NS * R], BF16, tag="mask_all",
                                 name="mask_all")
            nc.sync.dma_start(out=mask_all[:].rearrange("p (s t) -> p s t", s=NS),
                              in_=d_mask[:].rearrange("(s p) t -> p s t", p=P))
            mask_sb = [mask_all[:, i * R:(i + 1) * R] for i in range(NS)]
            xo32_all = pers.tile([P, NK * R], F32, tag="xo32_all",
                                 name="xo32_all")
            nc.sync.dma_start(out=xo32_all[:].rearrange("p (i r) -> p i r", i=NK),
                              in_=d_xown32[:].rearrange("(i p) r -> p i r", p=P))
            xown32_sb = [xo32_all[:, i * R:(i + 1) * R] for i in range(NK)]
            xobf_all = pers.tile([P, NK * R], BF16, tag="xobf_all",
                                 name="xobf_all")
            nc.sync.dma_start(out=xobf_all[:].rearrange("p (i r) -> p i r", i=NK),
                              in_=d_xownbf[:].rearrange("(i p) r -> p i r", p=P))
            xownbf_sb = [xobf_all[:, i * R:(i + 1) * R] for i in range(NK)]
            wkv_all = pers.tile([P, NK * L], BF16, tag="wkv_all",
                                name="wkv_all")
            nc.sync.dma_start(out=wkv_all[:].rearrange("p (i l) -> p i l", i=NK),
                              in_=d_wkv[:].rearrange("(i p) l -> p i l", p=P))
            wkv_sb = [wkv_all[:, i * L:(i + 1) * L] for i in range(NK)]
            wkT_all = pers.tile([Dh, H * L], BF16, tag="wkT_all",
                                name="wkT_all")
            nc.sync.dma_start(out=wkT_all[:].rearrange("p (h l) -> p h l", h=H),
                              in_=d_wkT[:].rearrange("h p l -> p h l"))
            wkT_sb = [wkT_all[:, h * L:(h + 1) * L] for h in range(H)]
            wv_all = pers.tile([L, H * Dh], BF16, tag="wv_all", name="wv_all")
            nc.sync.dma_start(out=wv_all[:].rearrange("l (h d) -> l h d", h=H),
                              in_=d_wv[:].rearrange("h l d -> l h d"))
            wv_sb = [wv_all[:, h * Dh:(h + 1) * Dh] for h in range(H)]

            # ---------- rstd for own chunk (q-side scale) ----------
            ss_own = pp.tile([1, R], F32, tag="qp7", name="ss_own")
            for ki in range(NK):
                sq = work.tile([P, R], BF16, tag="sq32", name="sq32", bufs=1)
                nc.vector.tensor_mul(sq[:], xown32_sb[ki], xown32_sb[ki])
                nc.tensor.matmul(ss_own[:], ones_col_bf[:], sq[:],
                                 start=(ki == 0), stop=(ki == NK - 1))
            tmp_rms = work.tile([1, R], F32, tag="tmpf", name="tmp_rms")
            nc.scalar.activation(tmp_rms[:], ss_own[:], AF.Sqrt,
                                 bias=eps1[:], scale=1.0 / C)
            scale_q = pers.tile([1, R], F32, tag="scale_q", name="scale_q")
            nc.vector.reciprocal(scale_q[:], tmp_rms[:])
            scale_q_bf = pers.tile([1, R], BF16, tag="scale_q_bf",
                                   name="scale_q_bf")
            nc.vector.tensor_scalar_mul(scale_q_bf[:], scale_q[:],
                                        1.0 / float(np.sqrt(Dh)))
            bc_ps = pp.tile([P, R], F32, tag="qp5", name="bc_ps")
            nc.tensor.matmul(bc_ps[:], ones_row_bf[:], scale_q_bf[:],
                             start=True, stop=True)
            scaleq_bc = pers.tile([P, R], F32, tag="scaleq_bc",
                                  name="scaleq_bc")
            nc.vector.tensor_copy(scaleq_bc[:], bc_ps[:])

            # ---------- z (both layouts) + rstd_full ----------
            zT_ps = [pp.tile([L, R], F32, tag=f"qp{nt}", name=f"zT{nt}")
                     for nt in range(4)]
            ssf_ps = [pp.tile([1, R], F32, tag=f"qp{4+nt}", name=f"ssf{nt}")
                      for nt in range(4)]
            for ki in range(NK):
                xf = work.tile([P, T], BF16, tag="xf", name="xf")
                nc.gpsimd.dma_start(out=xf[:],
                                    in_=d_xfull[ki * P:(ki + 1) * P, :])
                sqf = work.tile([P, T], BF16, tag="sqf", name="sqf")
                nc.vector.tensor_mul(sqf[:], xf[:], xf[:])
                for nt in range(4):
                    nc.tensor.matmul(zT_ps[nt][:], wkv_sb[ki],
                                     xf[:, nt * R:(nt + 1) * R],
                                     start=(ki == 0), stop=(ki == NK - 1))
                for nt in range(4):
                    nc.tensor.matmul(ssf_ps[nt][:], ones_col_bf[:],
                                     sqf[:, nt * R:(nt + 1) * R],
                                     start=(ki == 0), stop=(ki == NK - 1))
            # rstd_full row (1, T) -> per-partition columns (128, NS)
            rstf = pers.tile([1, T], F32, tag="rstf", name="rstf")
            for nt in range(4):
                tmpf = work.tile([1, R], F32, tag="tmpf", name="tmpf")
                nc.scalar.activation(tmpf[:], ssf_ps[nt][:], AF.Sqrt,
                                     bias=eps1[:], scale=1.0 / C)
                nc.vector.reciprocal(rstf[:, nt * R:(nt + 1) * R], tmpf[:])
            rstd_cols = pers.tile([P, NS], F32, tag="rstd_cols",
                                  name="rstd_cols")
            nc.sync.dma_start(
                out=rstd_cols[:],
                in_=rstf[:].rearrange("o (s p) -> p (o s)", p=P))
            # z_T unscaled (rstd folded into exp); z_aug scaled via transpose
            zT_sb = []
            for nt in range(4):
                zt = pers.tile([L, R], BF16, tag=f"zT_sb{nt}",
                               name=f"zT_sb{nt}")
                nc.vector.tensor_copy(zt[:], zT_ps[nt][:])
                zT_sb.append(zt)
            z_aug = []
            for si in range(NS):
                za = pers.tile([P, L + 1], BF16, tag=f"zaug{si}",
                               name=f"zaug{si}")
                nc.vector.memset(za[:, L:L + 1], 1.0)
                tp = pp.tile([P, L], BF16, tag="qp6", name="ztp")
                nt, j = si // 4, si % 4
                nc.tensor.transpose(tp[:], zT_sb[nt][:, j * P:(j + 1) * P],
                                    ident[0:L, 0:L])
                nc.scalar.activation(za[:, 0:L], tp[:], AF.Copy,
                                     scale=rstd_cols[:, si:si + 1])
                z_aug.append(za)

            # ---------- q projection (q_T = wq'.T @ xown) ----------
            q_all = pers.tile([P, NK * R], BF16, tag="q_all", name="q_all")
            q_sb = [q_all[:, mi * R:(mi + 1) * R] for mi in range(NK)]
            for half in range(2):
                qp = [pp.tile([P, R], F32, tag=f"qp{j}", name=f"qp{j}")
                      for j in range(8)]
                for ki in range(NK):
                    ws = work.tile([P, 8 * P], BF16, tag="wstripe",
                                   name="wstripe", bufs=4)
                    nc.sync.dma_start(
                        out=ws[:],
                        in_=d_wq[ki * P:(ki + 1) * P,
                                 half * 1024:(half + 1) * 1024])
                    for j in range(8):
                        nc.tensor.matmul(qp[j][:], ws[:, j * P:(j + 1) * P],
                                         xownbf_sb[ki],
                                         start=(ki == 0), stop=(ki == NK - 1))
                for j in range(8):
                    nc.vector.tensor_copy(q_sb[half * 8 + j], qp[j][:])

            # ---------- attention: 4 groups x 4 heads ----------
            y_all = pers.tile([P, H * R], BF16, tag="y_all", name="y_all")
            y_sb = [y_all[:, h * R:(h + 1) * R] for h in range(H)]
            for g in range(4):
                heads = [4 * g + i for i in range(4)]
                qls = []
                for i, h in enumerate(heads):
                    qlp = pp.tile([L, R], F32, tag=f"qp{4+i}", name="qlp")
                    nc.tensor.matmul(qlp[:], wkT_sb[h], q_sb[h],
                                     start=True, stop=True)
                    ql = work.tile([L, R], BF16, tag=f"ql{i}", name=f"ql{i}", bufs=1)
                    nc.vector.tensor_mul(ql[:], qlp[:], scaleq_bc[0:L, :])
                    qls.append(ql)
                s_ps = [pp.tile([L + 1, R], F32, tag=f"qp{i}", name="s_aug")
                        for i in range(4)]
                for si in range(NS):
                    nt, j = si // 4, si % 4
                    zts = zT_sb[nt][:, j * P:(j + 1) * P]
                    exps = []
                    for i in range(4):
                        a_ps = pp.tile([P, R], F32, tag=f"qp{4+i}",
                                       name="att")
                        nc.tensor.matmul(a_ps[:], zts, qls[i][:],
                                         start=True, stop=True)
                        ex = work.tile([P, R], BF16, tag=f"exp{si % 4}",
                                       name=f"exp{si % 4}", bufs=2)
                        nc.scalar.activation(ex[:], a_ps[:], AF.Exp,
                                             scale=rstd_cols[:, si:si + 1])
                        nc.vector.tensor_mul(ex[:], ex[:], mask_sb[si])
                        exps.append(ex)
                    for i in range(4):
                        nc.tensor.matmul(s_ps[i][:], z_aug[si][:],
                                         exps[i][:],
                                         start=(si == 0), stop=(si == NS - 1),
                                         skip_group_check=True)
                for i, h in enumerate(heads):
                    rd = work.tile([1, R], BF16, tag="rd", name="rd")
                    with nc.allow_low_precision(reason="bf16 softmax denom"):
                        nc.vector.reciprocal(rd[:], s_ps[i][L:L + 1, :])
                    rb_ps = pp.tile([L, R], F32, tag=f"qp{4+i}", name="rb_ps")
                    nc.tensor.matmul(rb_ps[:], ones_row_bf[0:1, 0:L], rd[:],
                                     start=True, stop=True)
                    rb = work.tile([L, R], BF16, tag=f"rb{i}", name=f"rb{i}", bufs=1)
                    nc.vector.tensor_copy(rb[:], rb_ps[:])
                    s_sb = work.tile([L, R], BF16, tag=f"s_sb{i}",
                                     name=f"s_sb{i}", bufs=1)
                    nc.vector.tensor_mul(s_sb[:], s_ps[i][0:L, :], rb[:])
                    y_ps = pp.tile([P, R], F32, tag=f"qp{4+i}", name="y_ps")
                    nc.tensor.matmul(y_ps[:], wv_sb[h], s_sb[:],
                                     start=True, stop=True)
                    nc.vector.tensor_copy(y_sb[h], y_ps[:])

            # ---------- o-proj + residual -> x2 (transposed, in xown32) ----
            for half in range(2):
                op = [pp.tile([P, R], F32, tag=f"qp{j}", name=f"qp{j}")
                      for j in range(8)]
                for ki in range(NK):
                    ws = work.tile([P, 8 * P], BF16, tag="wstripe",
                                   name="wstripe", bufs=4)
                    nc.sync.dma_start(
                        out=ws[:],
                        in_=d_wo[ki * P:(ki + 1) * P,
                                 half * 1024:(half + 1) * 1024])
                    for j in range(8):
                        nc.tensor.matmul(op[j][:], ws[:, j * P:(j + 1) * P],
                                         y_sb[ki],
                                         start=(ki == 0), stop=(ki == NK - 1))
                for j in range(8):
                    mi = half * 8 + j
                    nc.vector.tensor_add(xown32_sb[mi], op[j][:],
                                         xown32_sb[mi])

            # ---------- x2 -> natural layout + bf16 cast + rstd2 ----------
            x2nat = [pers.tile([P, C], BF16, tag=f"x2n{t}", name=f"x2n{t}")
                     for t in range(4)]
            for ki in range(NK):
                for t in range(4):
                    tp2 = pp.tile([P, P], F32, tag=f"qp{4 + t}", name="x2tp")
                    nc.tensor.transpose(
                        tp2[:], xown32_sb[ki][:, t * P:(t + 1) * P],
                        identf[:])
                    nc.vector.tensor_copy(x2nat[t][:, ki * P:(ki + 1) * P],
                                          tp2[:])
            x2bf_all = pers.tile([P, NK * R], BF16, tag="xobf_all",
                                 name="x2bf_all")
            x2bf_sb = [x2bf_all[:, i * R:(i + 1) * R] for i in range(NK)]
            for ki in range(NK):
                nc.vector.tensor_copy(x2bf_sb[ki], xown32_sb[ki])
            rstd2c = []
            for t in range(4):
                sqs = work.tile([P, C], BF16, tag="sqscr", name="sqscr", bufs=1)
                ssc = work.tile([P, 1], F32, tag="ssc", name="ssc")
                nc.scalar.activation(sqs[:], x2nat[t][:], AF.Square,
                                     accum_out=ssc[:])
                rc = pers.tile([P, 1], F32, tag=f"rstd2c{t}",
                               name=f"rstd2c{t}")
                nc.scalar.activation(rc[:], ssc[:], AF.Sqrt,
                                     bias=epsc[:], scale=1.0 / C)
                nc.vector.reciprocal(rc[:], rc[:])
                rstd2c.append(rc)

            # ---------- MLP ----------
            # fc = relu(wfc'.T @ x2bf): weight-stationary; fcT resident in
            # recycled attention tags (mask/exp/q/y are all dead by now).
            fcT_a = pers.tile([P, 16 * R], BF16, tag="mask_all",
                              name="fcT_a")
            fcT_b = pers.tile([P, 32 * R], BF16, tag="xo32_all",
                              name="fcT_b")
            fcT_c = pers.tile([P, 16 * R], BF16, tag="q_all", name="fcT_c")
            def _fcT(fi):
                if fi < 16:
                    return fcT_a[:, fi * R:(fi + 1) * R]
                if fi < 48:
                    return fcT_b[:, (fi - 16) * R:(fi - 15) * R]
                return fcT_c[:, (fi - 48) * R:(fi - 47) * R]
            fcT = [_fcT(fi) for fi in range(NF)]
            for fw in range(8):
                fp = [pp.tile([P, R], F32, tag=f"qp{j}", name=f"qp{j}")
                      for j in range(8)]
                for ki in range(NK):
                    ws = work.tile([P, 8 * P], BF16, tag="wstripe",
                                   name="wstripe", bufs=4)
                    _dma_eng = nc.sync if ki % 2 == 0 else nc.gpsimd
                    _dma_eng.dma_start(
                        out=ws[:],
                        in_=d_wfc[ki * P:(ki + 1) * P,
                                  fw * 1024:(fw + 1) * 1024])
                    for j in range(8):
                        nc.tensor.matmul(fp[j][:], ws[:, j * P:(j + 1) * P],
                                         x2bf_sb[ki],
                                         start=(ki == 0), stop=(ki == NK - 1))
                for j in range(8):
                    nc.scalar.activation(fcT[fw * 8 + j], fp[j][:],
                                         AF.Relu)
            # proj: activation-stationary (lhsT = fcT tiles), out natural.
            for chalf in range(2):
                mp = [[pp.tile([P, R], F32, tag=f"qp{t * 2 + c2}",
                               name=f"mp{t}{c2}")
                       for c2 in range(2)] for t in range(4)]
                for ki in range(NF):
                    ws = work.tile([P, 8 * P], BF16, tag="wstripe",
                                   name="wstripe", bufs=4)
                    _dma_eng = nc.sync if ki % 2 == 0 else nc.gpsimd
                    _dma_eng.dma_start(
                        out=ws[:],
                        in_=d_wproj[ki * P:(ki + 1) * P,
                                    chalf * 1024:(chalf + 1) * 1024])
                    for t in range(4):
                        for c2 in range(2):
                            nc.tensor.matmul(
                                mp[t][c2][:],
                                fcT[ki][:, t * P:(t + 1) * P],
                                ws[:, c2 * R:(c2 + 1) * R],
                                start=(ki == 0), stop=(ki == NF - 1))
                for t in range(4):
                    for c2 in range(2):
                        col = chalf * 1024 + c2 * R
                        ot = work.tile([P, R], F32, tag="ot",
                                       name="ot", bufs=2)
                        nc.vector.scalar_tensor_tensor(
                            ot[:],
                            mp[t][c2][:], rstd2c[t][:],
                            x2nat[t][:, col:col + R],
                            op0=mybir.AluOpType.mult,
                            op1=mybir.AluOpType.add)
                        nc.sync.dma_start(
                            out=d_out[t * P:(t + 1) * P, col:col + R],
                            in_=ot[:])

    nc.compile()
    return nc


def make_inputs(x, wq, w_kv, Wk_up, Wv_up, wo, ln1_w, ln2_w, w_fc, w_proj):
    """Host-side preprocessing -> per-core input maps."""
    x = np.asarray(x, np.float32)
    ln1 = np.asarray(ln1_w, np.float32)[:, None]
    ln2 = np.asarray(ln2_w, np.float32)[:, None]
    wq_bf = (ln1 * np.asarray(wq, np.float32)).astype(nbf)
    wkv_bf = (ln1 * np.asarray(w_kv, np.float32)).astype(nbf)
    wkT_bf = np.ascontiguousarray(
        np.asarray(Wk_up, np.float32).transpose(0, 2, 1)).astype(nbf)
    wv_bf = np.asarray(Wv_up, np.float32).astype(nbf)
    wo_bf = np.asarray(wo, np.float32).astype(nbf)
    wfc_bf = (ln2 * np.asarray(w_fc, np.float32)).astype(nbf)
    wproj_bf = np.asarray(w_proj, np.float32).astype(nbf)

    shared = dict(wq_bf=wq_bf, wkv_bf=wkv_bf, wkT_bf=wkT_bf, wv_bf=wv_bf,
                  wo_bf=wo_bf, wfc_bf=wfc_bf, wproj_bf=wproj_bf)
    in_maps = []
    for c in range(N_CORES):
        b, k = c // 4, c % 4
        xb = x[b]                                   # (T, C)
        xfullT = np.ascontiguousarray(xb.T)         # (C, T)
        xown = xb[k * R:(k + 1) * R]                # (R, C)
        xownT = np.ascontiguousarray(xown.T)        # (C, R)
        s = np.arange(T)[:, None]
        t = k * R + np.arange(R)[None, :]
        mask = (s <= t).astype(nbf)                 # (T, R)
        m = dict(shared)
        m.update(
            xfull_bf=xfullT.astype(nbf),
            xown_f32=xownT.astype(np.float32),
            xown_bf=xownT.astype(nbf),
            mask_bf=mask,
        )
        in_maps.append(m)
    return in_maps


_NC_CACHE = {}


def kernel(**inputs):
    if "nc" not in _NC_CACHE:
        _NC_CACHE["nc"] = build_nc()
    nc = _NC_CACHE["nc"]
    in_maps = make_inputs(**inputs)
    res = run_bass_kernel_spmd(nc, in_maps, list(range(N_CORES)))
    out = np.empty((B, T, C), np.float32)
    for c in range(N_CORES):
        b, k = c // 4, c % 4
        out[b, k * R:(k + 1) * R, :] = res.results[c]["out"]
    return out


if __name__ == "__main__":
    import importlib.util
    spec = importlib.util.spec_from_file_location(
        "reference", "/root/problem/reference.py")
    ref = importlib.util.module_from_spec(spec)
    spec.loader.exec_module(ref)
    inputs = {k: np.asarray(v) for k, v in ref.setup_inputs().items()}
    got = kernel(**inputs)
    exp = np.asarray(ref.reference(**inputs))
    rel = np.linalg.norm(got - exp) / np.linalg.norm(exp)
    print("max abs err:", np.abs(got - exp).max(), "rel:", rel)
